# revision 5
# baseline (speedup 1.0000x reference)
"""Trainium2 Bass kernel for supervised contrastive loss over N=8192 rows.

Strategy (8-core SPMD, rows sharded 1024/core):
  - Per column chunk t (128 cols): simT[c, r] = emb_cols_t @ emb_rows.T via PE
    (bf16), exp(sim/T) on the scalar engine (bf16 out), diagonal zeroed by an
    off-diag mask multiply, then S_T[class, row] += onehot_colsT @ exp on PE.
    Classes partition the columns, so per-row total = sum_c S_T[c, r] and
    positive = S_T[label_r, r]; the HOST does those trivial reductions plus
    the -log()/mean, so the device tail is just a PSUM->DRAM DMA of S_T.
  - The diagonal's chunk position is made core-invariant by rotating each
    core's column-side data (embeddings and one-hots) by its row offset.
  - PSUM: two [128, 1536] sim tiles ping-pong (3 banks each) + two [100, 512]
    S accumulators (1 bank each) = 8 banks. Activations span 1536 elements
    (3 half-chunks) to amortize the fixed per-ACTIVATE access latency.
  - DMA: few large triggers spread across the idle engine queues so the
    first chunk's deps land fast and the rest streams behind compute.
"""

import numpy as np
import ml_dtypes

import concourse.tile as tile
from concourse import bacc, mybir
from concourse.bass_utils import run_bass_kernel_spmd

N, D, C = 8192, 128, 100
NCORES = 8
R = N // NCORES  # rows per core
NT = N // 128  # column chunks of 128
NH = NT * 2  # half-chunks ([128 cols, 512 rows] units)
TEMP = 0.07
F32 = mybir.dt.float32
BF16 = mybir.dt.bfloat16

_PROGRAM_CACHE = {}


def _build_program():
    nc = bacc.Bacc("TRN2", target_bir_lowering=False, debug=False, num_devices=NCORES)

    embT_cols = nc.dram_tensor("embT_cols", [D, N], BF16, kind="ExternalInput")
    embT_rows = nc.dram_tensor("embT_rows", [D, R], BF16, kind="ExternalInput")
    # ohc[p, t*C + c] = onehot[t*128 + p, c] (rotated column order)
    ohc = nc.dram_tensor("ohc", [128, NT * C], BF16, kind="ExternalInput")
    offdiag = nc.dram_tensor("offdiag", [128, 128], BF16, kind="ExternalInput")
    out = nc.dram_tensor("out", [C, R], F32, kind="ExternalOutput")

    # groups of 3 half-chunks (last group has 2)
    GW = 3
    NG = (NH + GW - 1) // GW

    def gslice(g):
        return range(g * GW, min((g + 1) * GW, NH))

    with tile.TileContext(nc) as tc:
        with (
            tc.tile_pool(name="consts", bufs=1) as consts,
            tc.tile_pool(name="spool", bufs=1, space="PSUM") as spool,
            tc.tile_pool(name="simpool", bufs=2, space="PSUM") as simpool,
            tc.tile_pool(name="exppool", bufs=2) as exppool,
            tc.tile_pool(name="fsb", bufs=1) as fsb,
        ):
            rows_sb = consts.tile([D, R], BF16, tag="rows")
            cols_sb = consts.tile([D, N], BF16, tag="cols")
            ohc_sb = consts.tile([128, NT * C], BF16, tag="ohc")
            offd_sb = consts.tile([128, 128], BF16, tag="offd")

            # Critical-path loads first, on separate queues so they run in
            # parallel: rows + first col chunks + first one-hots. The bulk
            # streams behind compute.
            nc.sync.dma_start(rows_sb[:], embT_rows[:, :])
            nc.scalar.dma_start(cols_sb[:, 0:1024], embT_cols[:, 0:1024])
            nc.gpsimd.dma_start(offd_sb[:], offdiag[:, :])
            nc.gpsimd.dma_start(ohc_sb[:, 0 : 16 * C], ohc[:, 0 : 16 * C])
            nc.sync.dma_start(cols_sb[:, 1024:4096], embT_cols[:, 1024:4096])
            nc.scalar.dma_start(cols_sb[:, 4096:8192], embT_cols[:, 4096:8192])
            nc.gpsimd.dma_start(ohc_sb[:, 16 * C :], ohc[:, 16 * C :])

            # Preload the Exp activation table while DMA streams so the first
            # real activation doesn't pay the table switch.
            warm = fsb.tile([1, 1], F32, tag="warm")
            nc.vector.memset(warm[:], 0.0)
            warm_out = fsb.tile([1, 1], F32, tag="warm_out")
            nc.scalar.activation(
                warm_out[:], warm[:], mybir.ActivationFunctionType.Exp
            )

            # S_T[class, row] accumulators over all column chunks; one PSUM
            # bank per 512-row half (a matmul output must stay in one bank).
            S_T = [
                spool.tile([C, 512], F32, tag=f"S{q}", name=f"S_T{q}") for q in range(2)
            ]

            sim_of_group = [None] * NG
            exp_of_group = [None] * NG

            def emit_sim(g):
                hs = gslice(g)
                w = 512 * len(hs)
                sim_ps = simpool.tile([128, w], F32, name=f"sim{g}", tag="sim")
                for i, h in enumerate(hs):
                    tt, q = h // 2, h % 2
                    nc.tensor.matmul(
                        sim_ps[:, i * 512 : (i + 1) * 512],
                        cols_sb[:, tt * 128 : (tt + 1) * 128],
                        rows_sb[:, q * 512 : (q + 1) * 512],
                        start=True,
                        stop=True,
                    )
                sim_of_group[g] = sim_ps

            def emit_exp(g):
                hs = gslice(g)
                w = 512 * len(hs)
                exp_sb = exppool.tile([128, w], BF16, name=f"exp{g}", tag="exp")
                nc.scalar.activation(
                    exp_sb[:],
                    sim_of_group[g][:],
                    mybir.ActivationFunctionType.Exp,
                    scale=float(1.0 / TEMP),
                )
                # Zero the diagonal block: chunk tt < 8 holds this core's own
                # rows as columns; its diagonal block covers chunk-rows
                # [tt*128, tt*128+128) which live in half h = 2*tt + (tt>=4).
                for i, h in enumerate(hs):
                    tt, q = h // 2, h % 2
                    if tt < 8 and (tt * 128) // 512 == q:
                        off = i * 512 + (tt * 128) % 512
                        nc.vector.tensor_mul(
                            exp_sb[:, off : off + 128],
                            exp_sb[:, off : off + 128],
                            offd_sb[:],
                        )
                exp_of_group[g] = exp_sb

            def emit_accum(g):
                hs = gslice(g)
                for i, h in enumerate(hs):
                    tt, q = h // 2, h % 2
                    nc.tensor.matmul(
                        S_T[q][:],
                        ohc_sb[:, tt * C : (tt + 1) * C],
                        exp_of_group[g][:, i * 512 : (i + 1) * 512],
                        start=(tt == 0),
                        stop=(tt == NT - 1),
                    )

            emit_sim(0)
            emit_sim(1)
            for g in range(NG):
                if g + 2 < NG:
                    emit_sim(g + 2)
                emit_exp(g)
                emit_accum(g)

            # Tail: stage the accumulated S tables to SBUF, then DMA out.
            S_sb = fsb.tile([C, R], F32, tag="S_sb")
            for q in range(2):
                nc.vector.tensor_copy(S_sb[:, q * 512 : (q + 1) * 512], S_T[q][:])
                qeng = nc.sync if q == 0 else nc.gpsimd
                qeng.dma_start(
                    out[:, q * 512 : (q + 1) * 512], S_sb[:, q * 512 : (q + 1) * 512]
                )

    nc.compile()
    return nc


def _get_program():
    if "nc" not in _PROGRAM_CACHE:
        _PROGRAM_CACHE["nc"] = _build_program()
    return _PROGRAM_CACHE["nc"]


def _prepare_in_maps(embeddings, labels):
    emb = np.asarray(embeddings, dtype=np.float32)
    lab = np.asarray(labels).astype(np.int64)
    embT = np.ascontiguousarray(emb.T).astype(ml_dtypes.bfloat16)  # [D, N]
    classes = np.arange(C, dtype=np.int64)
    onehot = (lab[:, None] == classes[None, :]).astype(ml_dtypes.bfloat16)  # [N, C]
    offd = (1.0 - np.eye(128, dtype=np.float32)).astype(ml_dtypes.bfloat16)

    in_maps = []
    for i in range(NCORES):
        r0 = i * R
        oh_rot = np.roll(onehot, -r0, axis=0)  # [N, C]
        # [128, NT*C]: line p = concat over t of onehot[t*128 + p, :]
        ohc_pt = np.ascontiguousarray(
            oh_rot.reshape(NT, 128, C).transpose(1, 0, 2).reshape(128, NT * C)
        )
        in_maps.append(
            {
                "embT_cols": np.ascontiguousarray(np.roll(embT, -r0, axis=1)),
                "embT_rows": np.ascontiguousarray(embT[:, r0 : r0 + R]),
                "ohc": ohc_pt,
                "offdiag": offd,
            }
        )
    return in_maps, lab


def run(embeddings, labels, trace=False, trace_cores=None):
    """Returns (mean_loss, BassKernelResults)."""
    nc = _get_program()
    in_maps, lab = _prepare_in_maps(embeddings, labels)
    kwargs = {}
    if trace:
        kwargs["trace"] = True
        if trace_cores is not None:
            kwargs["trace_cores"] = trace_cores
    res = run_bass_kernel_spmd(nc, in_maps, core_ids=list(range(NCORES)), **kwargs)

    S = np.concatenate(
        [np.asarray(res.results[i]["out"], dtype=np.float64) for i in range(NCORES)],
        axis=1,
    )  # [C, N]
    total = S.sum(axis=0)  # [N]
    pos = S[lab, np.arange(N)]  # [N]
    counts = np.bincount(lab, minlength=C)
    valid = (counts[lab] - 1) > 0
    loss = -np.log(pos / (total + 1e-8) + 1e-8)
    cnt = int(valid.sum())
    mean = float(loss[valid].sum() / cnt) if cnt > 0 else 0.0
    return np.asarray(mean, dtype=np.float32), res


def kernel(embeddings, labels):
    return run(embeddings, labels)[0]


# revision 7
# speedup vs baseline: 1.1588x; 1.1588x over previous
"""Trainium2 Bass kernel for supervised contrastive loss over N=8192 rows.

Strategy (8-core SPMD, rows sharded 1024/core):
  - Per column chunk t (128 cols): simT[c, r] = emb_cols_t @ emb_rows.T via PE
    (bf16), exp(sim/T) on the scalar engine (bf16 out), diagonal zeroed by an
    off-diag mask multiply, then S_T[class, row] += onehot_colsT @ exp on PE.
    Classes partition the columns, so per-row total = sum_c S_T[c, r] and
    positive = S_T[label_r, r]; the HOST does those trivial reductions plus
    the -log()/mean, so the device tail is just staging S_T out.
  - The diagonal's chunk position is made core-invariant by rotating each
    core's column-side data (embeddings and one-hots) by its row offset.
  - PSUM: two [128, 1536] sim tiles ping-pong (3 banks each) + two [100, 512]
    S accumulators (1 bank each) = 8 banks. Activations span up to 1536
    elements (3 half-chunks) to amortize the per-ACTIVATE access latency;
    the first two groups are 1 and 2 halves so the scalar engine starts as
    early as possible.
  - Ramp: critical-path DMA split small and spread over the sync + gpsimd
    queues (rows halves first, then col chunks in increasing-need order);
    a short chain of dummy matmuls keeps the PE out of its cold p-state so
    the first real sims run at full clock.
"""

import numpy as np
import ml_dtypes

import concourse.tile as tile
from concourse import bacc, mybir
from concourse.bass_utils import run_bass_kernel_spmd

N, D, C = 8192, 128, 100
NCORES = 8
R = N // NCORES  # rows per core
NT = N // 128  # column chunks of 128
NH = NT * 2  # half-chunks ([128 cols, 512 rows] units)
TEMP = 0.07
F32 = mybir.dt.float32
BF16 = mybir.dt.bfloat16

_PROGRAM_CACHE = {}

# group sizes in half-chunks: open small so the scalar engine starts early
_GROUPS = [1, 2] + [3] * ((NH - 5) // 3) + [2]
assert sum(_GROUPS) == NH


def _gslice(g):
    s = sum(_GROUPS[:g])
    return range(s, s + _GROUPS[g])


def _build_program():
    nc = bacc.Bacc("TRN2", target_bir_lowering=False, debug=False, num_devices=NCORES)

    embT_cols = nc.dram_tensor("embT_cols", [D, N], BF16, kind="ExternalInput")
    embT_rows = nc.dram_tensor("embT_rows", [D, R], BF16, kind="ExternalInput")
    # ohc[p, t*C + c] = onehot[t*128 + p, c] (rotated column order)
    ohc = nc.dram_tensor("ohc", [128, NT * C], BF16, kind="ExternalInput")
    offdiag = nc.dram_tensor("offdiag", [128, 128], BF16, kind="ExternalInput")
    out = nc.dram_tensor("out", [C, R], BF16, kind="ExternalOutput")

    NG = len(_GROUPS)

    with tile.TileContext(nc) as tc:
        with (
            tc.tile_pool(name="consts", bufs=1) as consts,
            tc.tile_pool(name="spool", bufs=1, space="PSUM") as spool,
            tc.tile_pool(name="simpool", bufs=2, space="PSUM") as simpool,
            tc.tile_pool(name="exppool", bufs=3) as exppool,
            tc.tile_pool(name="fsb", bufs=1) as fsb,
        ):
            rows_sb = consts.tile([D, R], BF16, tag="rows")
            cols_sb = consts.tile([D, N], BF16, tag="cols")
            ohc_sb = consts.tile([128, NT * C], BF16, tag="ohc")
            offd_sb = consts.tile([128, 128], BF16, tag="offd")

            # Critical-path loads first, split small across the two DMA-
            # capable idle queues so the first chunks' deps land fast; the
            # bulk streams behind compute.
            nc.sync.dma_start(rows_sb[:, 0:512], embT_rows[:, 0:512])
            nc.gpsimd.dma_start(rows_sb[:, 512:1024], embT_rows[:, 512:1024])
            nc.sync.dma_start(cols_sb[:, 0:256], embT_cols[:, 0:256])
            nc.gpsimd.dma_start(offd_sb[:], offdiag[:, :])
            nc.sync.dma_start(cols_sb[:, 256:1024], embT_cols[:, 256:1024])
            nc.gpsimd.dma_start(ohc_sb[:, 0 : 8 * C], ohc[:, 0 : 8 * C])
            nc.sync.dma_start(cols_sb[:, 1024:2048], embT_cols[:, 1024:2048])
            nc.gpsimd.dma_start(ohc_sb[:, 8 * C : 24 * C], ohc[:, 8 * C : 24 * C])
            nc.sync.dma_start(cols_sb[:, 2048:4096], embT_cols[:, 2048:4096])
            nc.sync.dma_start(cols_sb[:, 4096:8192], embT_cols[:, 4096:8192])
            nc.gpsimd.dma_start(ohc_sb[:, 24 * C :], ohc[:, 24 * C :])

            # Preload the Exp activation table while DMA streams so the first
            # real activation doesn't pay the table switch.
            warm = fsb.tile([1, 1], F32, tag="warm")
            nc.vector.memset(warm[:], 0.0)
            warm_out = fsb.tile([1, 1], F32, tag="warm_out")
            nc.scalar.activation(
                warm_out[:], warm[:], mybir.ActivationFunctionType.Exp
            )

            # Dummy matmul chain on a zero tile: keeps the PE clock ramping
            # during the DMA wait so the first real sims run at full p-state.
            # Uses the S0-tagged slot, which the real S accumulator only
            # needs once the first exp tile is ready.
            zeros_sb = fsb.tile([128, 512], BF16, tag="zeros")
            nc.vector.memset(zeros_sb[:], 0.0)
            pewarm = spool.tile([128, 512], F32, tag="S0", name="pewarm")
            for _ in range(6):
                nc.tensor.matmul(
                    pewarm[:], zeros_sb[:, 0:128], zeros_sb[:], start=True, stop=True
                )

            # S_T[class, row] accumulators over all column chunks; one PSUM
            # bank per 512-row half (a matmul output must stay in one bank).
            S_T = [
                spool.tile([C, 512], F32, tag=f"S{q}", name=f"S_T{q}") for q in range(2)
            ]

            sim_of_group = [None] * NG
            exp_of_group = [None] * NG

            def emit_sim(g):
                hs = _gslice(g)
                w = 512 * len(hs)
                sim_ps = simpool.tile([128, w], F32, name=f"sim{g}", tag="sim")
                for i, h in enumerate(hs):
                    tt, q = h // 2, h % 2
                    nc.tensor.matmul(
                        sim_ps[:, i * 512 : (i + 1) * 512],
                        cols_sb[:, tt * 128 : (tt + 1) * 128],
                        rows_sb[:, q * 512 : (q + 1) * 512],
                        start=True,
                        stop=True,
                    )
                sim_of_group[g] = sim_ps

            def emit_exp(g):
                hs = _gslice(g)
                w = 512 * len(hs)
                exp_sb = exppool.tile([128, w], BF16, name=f"exp{g}", tag="exp")
                nc.scalar.activation(
                    exp_sb[:],
                    sim_of_group[g][:],
                    mybir.ActivationFunctionType.Exp,
                    scale=float(1.0 / TEMP),
                )
                # Zero the diagonal block: chunk tt < 8 holds this core's own
                # rows as columns; its diagonal block covers chunk-rows
                # [tt*128, tt*128+128) which live in half h = 2*tt + (tt>=4).
                for i, h in enumerate(hs):
                    tt, q = h // 2, h % 2
                    if tt < 8 and (tt * 128) // 512 == q:
                        off = i * 512 + (tt * 128) % 512
                        nc.vector.tensor_mul(
                            exp_sb[:, off : off + 128],
                            exp_sb[:, off : off + 128],
                            offd_sb[:],
                        )
                exp_of_group[g] = exp_sb

            def emit_accum(g):
                hs = _gslice(g)
                for i, h in enumerate(hs):
                    tt, q = h // 2, h % 2
                    nc.tensor.matmul(
                        S_T[q][:],
                        ohc_sb[:, tt * C : (tt + 1) * C],
                        exp_of_group[g][:, i * 512 : (i + 1) * 512],
                        start=(tt == 0),
                        stop=(tt == NT - 1),
                    )

            emit_sim(0)
            emit_sim(1)
            for g in range(NG):
                if g + 2 < NG:
                    emit_sim(g + 2)
                emit_exp(g)
                emit_accum(g)

            # Tail: stage S to SBUF as bf16 (DVE cast) and DMA out on the two
            # HWDGE queues in parallel.
            S_sb = fsb.tile([C, R], BF16, tag="S_sb")
            for q in range(2):
                nc.vector.tensor_copy(S_sb[:, q * 512 : (q + 1) * 512], S_T[q][:])
            nc.sync.dma_start(out[:, 0:512], S_sb[:, 0:512])
            nc.scalar.dma_start(out[:, 512:1024], S_sb[:, 512:1024])

    nc.compile()
    return nc


def _get_program():
    if "nc" not in _PROGRAM_CACHE:
        _PROGRAM_CACHE["nc"] = _build_program()
    return _PROGRAM_CACHE["nc"]


def _prepare_in_maps(embeddings, labels):
    emb = np.asarray(embeddings, dtype=np.float32)
    lab = np.asarray(labels).astype(np.int64)
    embT = np.ascontiguousarray(emb.T).astype(ml_dtypes.bfloat16)  # [D, N]
    classes = np.arange(C, dtype=np.int64)
    onehot = (lab[:, None] == classes[None, :]).astype(ml_dtypes.bfloat16)  # [N, C]
    offd = (1.0 - np.eye(128, dtype=np.float32)).astype(ml_dtypes.bfloat16)

    in_maps = []
    for i in range(NCORES):
        r0 = i * R
        oh_rot = np.roll(onehot, -r0, axis=0)  # [N, C]
        # [128, NT*C]: line p = concat over t of onehot[t*128 + p, :]
        ohc_pt = np.ascontiguousarray(
            oh_rot.reshape(NT, 128, C).transpose(1, 0, 2).reshape(128, NT * C)
        )
        in_maps.append(
            {
                "embT_cols": np.ascontiguousarray(np.roll(embT, -r0, axis=1)),
                "embT_rows": np.ascontiguousarray(embT[:, r0 : r0 + R]),
                "ohc": ohc_pt,
                "offdiag": offd,
            }
        )
    return in_maps, lab


def run(embeddings, labels, trace=False, trace_cores=None):
    """Returns (mean_loss, BassKernelResults)."""
    nc = _get_program()
    in_maps, lab = _prepare_in_maps(embeddings, labels)
    kwargs = {}
    if trace:
        kwargs["trace"] = True
        if trace_cores is not None:
            kwargs["trace_cores"] = trace_cores
    res = run_bass_kernel_spmd(nc, in_maps, core_ids=list(range(NCORES)), **kwargs)

    S = np.concatenate(
        [np.asarray(res.results[i]["out"]).astype(np.float64) for i in range(NCORES)],
        axis=1,
    )  # [C, N]
    total = S.sum(axis=0)  # [N]
    pos = S[lab, np.arange(N)]  # [N]
    counts = np.bincount(lab, minlength=C)
    valid = (counts[lab] - 1) > 0
    loss = -np.log(pos / (total + 1e-8) + 1e-8)
    cnt = int(valid.sum())
    mean = float(loss[valid].sum() / cnt) if cnt > 0 else 0.0
    return np.asarray(mean, dtype=np.float32), res


def kernel(embeddings, labels):
    return run(embeddings, labels)[0]
